# revision 94
# baseline (speedup 1.0000x reference)
"""Single-head causal attention kernel for Trainium2 (8 NeuronCores).

Problem: x[8, 2048, 1024], Wq/Wk/Wv[1024, 64] ->
  out[b] = softmax(causal((x[b] @ Wq) @ (x[b] @ Wk)^T / 8)) @ (x[b] @ Wv)

Sharding: data-parallel over batch, one batch element per core, weights
replicated.

Design notes:
  - all matmul inputs bf16 (x/W converted on host): halves DMA bytes,
    full PE rate (1 cycle/row) incl. the 128-wide diagonal score
    matmuls that f32r would run at 1/4 rate
  - host prepacks x into [p, block, chunk, t] and W into [p, chunk, h]
    so each x 512-block is ONE dma (128 descriptors x 8KB contiguous);
    block 0 and the weights are split for an earlier start
  - a PE warmup matmul during the initial DMA wait ramps the tensor
    engine p-state; a dummy exp preloads the ACT function table
  - emission interleaves att(b) with proj(b+1): scores issue as soon as
    q/k copies land, v(b) completes just before att(b)'s diagonal PVs,
    qk(b+1) matmuls fill the PE while the scalar engine works through
    the exp chain (the secondary bottleneck), PV steps are exp-gated
  - scores computed transposed: ST_j = kT_j^T . qT, causal diag tile
    masked additively with -1e9, exp(0.125*ST) into bf16 PT
  - out^T[65, t-cols] accumulates v_aug_j^T . PT_j over s-chunks j
    (ones column appended to v makes row 64 the softmax denominator)
  - normalize: reciprocal of row 64 on DVE, partition-broadcast on the
    idle gpsimd engine, fused multiply reads the psum numerator
    directly (no copy); blocks 0-2 defer this into the next phase
  - six of block 3's off-diagonal s-chunks run early, at the end of the
    att(2) emission: their exps fill the scalar engine's idle seam at
    the phase boundary and their h0-PVs open the long-lived block-3
    psum; the h1-PVs are deferred via kept-alive PT tiles, shortening
    the ACT-bound endgame by several microseconds
  - block 3 closes as two 256-column halves whose j-sets complete at
    different times: the first half's normalize+DMA overlaps the second
    half's tail, and the final DMA issues from the idle ACT queue so it
    isn't serialized behind the first half's DMA on the SP queue
"""

import numpy as np
import ml_dtypes
from contextlib import ExitStack

import concourse.bass as bass
import concourse.tile as tile
import concourse.bacc as bacc
from concourse import mybir
from concourse import bass_utils
from concourse.masks import make_identity

F32 = mybir.dt.float32
BF16 = mybir.dt.bfloat16

T = 2048
C = 1024
H = 64
NCH = C // 128   # 8 contraction chunks
NB = T // 512    # 4 t-blocks
NEG = -1.0e9
N_WARMUP = 1      # one matmul suffices: the p-state ramp, once started,
                  # survives short pipeline gaps
FILLER_BIAS = 2   # projection filler front-bias (drain by PV nj-2)
EXP = mybir.ActivationFunctionType.Exp


def build_bass():
    nc = bacc.Bacc("TRN2", target_bir_lowering=False, debug=False, num_devices=8)
    xp = nc.dram_tensor("xp", [128, NB, NCH, 512], BF16, kind="ExternalInput").ap()
    wqk = nc.dram_tensor("wqk", [128, NCH, 128], BF16, kind="ExternalInput").ap()
    wv = nc.dram_tensor("wv", [128, NCH, 64], BF16, kind="ExternalInput").ap()
    outT = nc.dram_tensor("outT", [H, T], F32, kind="ExternalOutput").ap()

    with tile.TileContext(nc) as tc:
        with ExitStack() as ctx:
            build_kernel(ctx, tc, nc, xp, wqk, wv, outT)
    nc.compile()
    return nc


def build_kernel(ctx, tc, nc, xp, wqk, wv, outT):
    const = ctx.enter_context(tc.tile_pool(name="const", bufs=1))
    pt_pool = ctx.enter_context(tc.tile_pool(name="pt", bufs=14))
    vt_pool = ctx.enter_context(tc.tile_pool(name="vt", bufs=2))
    fin_pool = ctx.enter_context(tc.tile_pool(name="fin", bufs=4))
    rc_pool = ctx.enter_context(tc.tile_pool(name="rc", bufs=4))
    rb_pool = ctx.enter_context(tc.tile_pool(name="rb", bufs=4))
    qk_ps = ctx.enter_context(tc.tile_pool(name="qkps", bufs=1, space="PSUM"))
    v_ps = ctx.enter_context(tc.tile_pool(name="vps", bufs=1, space="PSUM"))
    st_ps = ctx.enter_context(tc.tile_pool(name="stps", bufs=4, space="PSUM"))
    o_ps = ctx.enter_context(tc.tile_pool(name="ops", bufs=2, space="PSUM"))

    # persistent sbuf state
    xt = const.tile([128, NB, NCH, 512], BF16)    # x chunks: [c-part, blk, chunk, t]
    w_qk = const.tile([128, NCH, 128], BF16)      # [Wq|Wk] per c-chunk
    w_v = const.tile([128, NCH, 64], BF16)        # Wv per c-chunk
    qT_sb = const.tile([64, T], BF16)
    kT_sb = const.tile([64, T], BF16)
    v_sb = const.tile([128, T // 128, H + 1], BF16)  # v natural + ones col
    neg_mask_f = const.tile([128, 128], F32)     # 0 where t>=s, -1e9 below diag
    id_sb = const.tile([64, 64], BF16)           # identity for PE transpose
    wu = const.tile([128, 512], BF16)            # warmup zeros

    # --- input DMAs first so transfers start immediately.  Block 0 is
    # split in two so the first proj chain can start earlier; wv is only
    # needed once the block-0 v chain runs mid-att(0).
    nc.sync.dma_start(xt[:, 0, 0:4, :], xp[:, 0, 0:4, :])
    nc.sync.dma_start(w_qk, wqk)
    nc.sync.dma_start(xt[:, 0, 4:8, :], xp[:, 0, 4:8, :])
    nc.sync.dma_start(w_v, wv)
    for b in range(1, NB):
        nc.sync.dma_start(xt[:, b], xp[:, b])

    # --- PE warmup (gpsimd memset is the fastest-available first writer)
    nc.gpsimd.memset(wu, 0.0)
    for i in range(N_WARMUP):
        wu_t = qk_ps.tile([128, 512], F32, tag="qk")
        nc.tensor.matmul(wu_t, wu[:, 0:128], wu, start=True, stop=True)
    # preload the ACT exp table during the DMA wait
    junk = const.tile([128, 4], BF16)
    nc.scalar.activation(junk, wu[:, 0:4], func=EXP, scale=0.125)

    # --- constants
    nc.gpsimd.memset(neg_mask_f, 0.0)
    nc.gpsimd.affine_select(
        out=neg_mask_f, in_=neg_mask_f, compare_op=mybir.AluOpType.is_ge,
        fill=NEG, base=0, pattern=[[1, 128]], channel_multiplier=-1,
    )
    make_identity(nc, id_sb)
    ones_f = const.tile([128, 64], F32)
    nc.vector.memset(ones_f, 1.0)
    for j in range(T // 128):
        nc.vector.tensor_copy(v_sb[:, j, H : H + 1], ones_f[:, 0:1])
    zeros_f = const.tile([128, 384], F32)
    nc.vector.memset(zeros_f, 0.0)
    # dedicated PT slots for diagonal s-chunks, one per within-block offset r:
    # the pad region [0:128r] is zeroed once here and never overwritten (exp
    # always writes exactly [128r:512]), so no per-tile re-padding is needed
    pt_diag1 = pt_pool.tile([128, 512], BF16, tag="ptd1")
    pt_diag2 = pt_pool.tile([128, 512], BF16, tag="ptd2")
    pt_diag3 = pt_pool.tile([128, 512], BF16, tag="ptd3")
    pt_diag = {1: pt_diag1, 2: pt_diag2, 3: pt_diag3}
    for r in range(1, 4):
        nc.vector.tensor_copy(pt_diag[r][:, 0 : 128 * r], zeros_f[:, 0 : 128 * r])

    def qk_steps(b):
        """q/k projection of block b as single-instruction emission steps."""
        blk = slice(512 * b, 512 * (b + 1))
        qk_t = qk_ps.tile([128, 512], F32, tag="qk")
        steps = []
        for j in range(NCH):
            steps.append(lambda j=j: nc.tensor.matmul(
                qk_t, w_qk[:, j, :], xt[:, b, j, :],
                start=(j == 0), stop=(j == NCH - 1), skip_group_check=True))

        def copies():
            nc.vector.tensor_copy(qT_sb[:, blk], qk_t[0:64, :])
            nc.vector.tensor_copy(kT_sb[:, blk], qk_t[64:128, :])
        steps.append(copies)
        return steps

    def v_steps(b):
        """v projection + transpose of block b; used as att(b) PE filler,
        guaranteed complete before att(b)'s first diagonal PV."""
        v_t = v_ps.tile([64, 512], F32)
        vt_s = vt_pool.tile([64, 512], BF16)
        steps = []
        for j in range(NCH):
            steps.append(lambda j=j: nc.tensor.matmul(
                v_t, w_v[:, j, :], xt[:, b, j, :],
                start=(j == 0), stop=(j == NCH - 1), skip_group_check=True))
        steps.append(lambda: nc.vector.tensor_copy(vt_s, v_t))
        for r in range(4):
            def tr(r=r):
                tp = st_ps.tile([128, 64], BF16, tag="st")
                nc.tensor.transpose(tp, vt_s[:, 128 * r : 128 * (r + 1)], id_sb)
                nc.vector.tensor_copy(v_sb[:, 4 * b + r, 0:H], tp)
            steps.append(tr)
        return steps

    def att_emit(b, vfill, qkfill, pre, last_ctx=None, pre_out=None,
                 skip_js=()):
        """Emit attention for block b.  `vfill` (v projection of b, must
        complete before the first diagonal PV) and `qkfill` (q/k projection
        of b+1) are spread between PV steps; `pre` is the previous block's
        deferred normalize (PE-free), emitted after the first scores."""
        last = (b == NB - 1)
        if last:
            # off-diag chunks 0,1 first (they need only the qT3 copy, not
            # kT3, so they start before the second boundary copy lands),
            # then diag 12,13 (early mask chains); trailing diag last --
            # they only feed the second half.  j=2,3 ran early at the end
            # of att(2) (scores+exp+h0-PV); only their h1-PVs remain here.
            L = [0, 1, 12, 13] + list(range(8, 12)) + [14, 15]
        else:
            L = [j for j in list(range(4 * b)) + [4 * b + r for r in range(4)]
                 if j not in skip_js]
        nj = len(L)
        pts = [None] * nj

        def score(i):
            j = L[i]
            r = j - 4 * b
            coff = 0 if r < 0 else 128 * r
            width = 512 - coff
            pt = pt_diag[r] if r > 0 else pt_pool.tile([128, 512], BF16)
            pts[i] = pt
            st = st_ps.tile([128, 512], F32, tag="st")
            kTj = kT_sb[:, 128 * j : 128 * (j + 1)]
            t0 = 512 * b + coff
            nc.tensor.matmul(st[:, 0:width], kTj, qT_sb[:, t0 : 512 * (b + 1)],
                             start=True, stop=True, skip_group_check=True)
            if r >= 0:
                nc.vector.tensor_add(st[:, 0:128], st[:, 0:128], neg_mask_f)
            nc.scalar.activation(pt[:, coff:512], st[:, 0:width],
                                 func=EXP, scale=0.125)

        filler = list(vfill) + list(qkfill)
        nvf = len(vfill)
        fi = 0

        for i in range(min(4, nj)):
            score(i)
        # the last block's deferred-normalize predecessor must run before
        # its very first PV (psum slot recycling); earlier blocks' can wait
        # until the diagonal drain, keeping the DVE queue clear for the
        # boundary q/k copies
        prestate = [pre]
        if last and pre is not None:
            pre()
            prestate[0] = None

        def fillers(i):
            nonlocal fi
            if L[i] >= 4 * b:          # diagonal PV: v(b) must be in place
                if prestate[0] is not None:
                    prestate[0]()
                    prestate[0] = None
                while fi < nvf:
                    filler[fi]()
                    fi += 1
            den = nj if b == 0 else max(1, nj - FILLER_BIAS
                                        - (2 if b == 2 else 0))
            want = min(len(filler), (len(filler) * (i + 1)) // den)
            while fi < want:
                filler[fi]()
                fi += 1

        if not last:
            blk = slice(512 * b, 512 * (b + 1))
            out_t = pre_out if pre_out is not None else \
                o_ps.tile([65, 512], F32, tag="o")
            for i in range(nj):
                fillers(i)
                nc.tensor.matmul(out_t, v_sb[:, L[i], :], pts[i],
                                 start=(i == 0 and pre_out is None),
                                 stop=(i == nj - 1),
                                 skip_group_check=True)
                if i + 4 < nj:
                    score(i + 4)
            while fi < len(filler):
                filler[fi]()
                fi += 1

            def normalize():
                rcf = rc_pool.tile([1, 512], F32)
                nc.vector.reciprocal(rcf, out_t[64:65, :])
                rb_sb = rb_pool.tile([64, 512], F32)
                nc.gpsimd.partition_broadcast(rb_sb, rcf)
                fin = fin_pool.tile([64, 512], F32)
                nc.vector.tensor_mul(fin, out_t[0:64, :], rb_sb)
                nc.sync.dma_start(outT[:, blk], fin)
            return normalize

        # ---- last block: two 256-column halves.  h0 (cols 1536:1792) only
        # needs j<=13; its normalize overlaps the rest.
        out_a, out_b, epts = last_ctx
        fin_a = fin_pool.tile([64, 256], F32)
        fin_b = fin_pool.tile([64, 256], F32)
        rc_a = rc_pool.tile([1, 256], F32)
        rc_b = rc_pool.tile([1, 256], F32)
        rb_a = rb_pool.tile([64, 256], F32)
        rb_b = rb_pool.tile([64, 256], F32)

        def pv(i):
            j = L[i]
            if j <= 13:
                nc.tensor.matmul(out_a, v_sb[:, j, :], pts[i][:, 0:256],
                                 start=False, stop=(i == 7),
                                 skip_group_check=True)
            nc.tensor.matmul(out_b, v_sb[:, j, :], pts[i][:, 256:512],
                             start=False, stop=(i == nj - 1),
                             skip_group_check=True)

        # deferred h1 contributions of the early units (exps long done)
        for jj, ept in epts:
            nc.tensor.matmul(out_b, v_sb[:, jj, :], ept[:, 256:512],
                             start=(jj == epts[0][0]), stop=False,
                             skip_group_check=True)
        for i in range(nj):
            fillers(i)
            pv(i)
            if i + 4 < nj:
                score(i + 4)
            if i == 7:
                # h0 closed: start its normalize; chain overlaps the rest.
                # Reciprocal first (Tile serializes same-tile readers), then
                # partition-broadcast on the idle gpsimd engine; the multiply
                # reads the psum numerator directly (no fin copy at all).
                nc.vector.reciprocal(rc_a, out_a[64:65, :])
                nc.gpsimd.partition_broadcast(rb_a, rc_a)
        while fi < len(filler):
            filler[fi]()
            fi += 1
        nc.vector.tensor_mul(fin_a, out_a[0:64, :], rb_a)
        nc.sync.dma_start(outT[:, 1536:1792], fin_a)
        nc.vector.reciprocal(rc_b, out_b[64:65, :])
        nc.gpsimd.partition_broadcast(rb_b, rc_b)
        nc.vector.tensor_mul(fin_b, out_b[0:64, :], rb_b)
        # final DMA issues from the (idle by now) ACT queue so its
        # descriptor generation isn't serialized behind the h0 DMA on SP
        nc.scalar.dma_start(outT[:, 1792:2048], fin_b)
        return None

    for s in qk_steps(0):
        s()
    pre = att_emit(0, v_steps(0), qk_steps(1), None)
    pre = att_emit(1, v_steps(1), qk_steps(2), pre)
    vf2 = v_steps(2)
    qf3 = qk_steps(3)
    pre = att_emit(2, vf2, qf3, pre)
    # early block-3 units (j=2,3): their scores+exps fill the ACT seam at
    # the att2/att3 boundary and their h0 PVs open out_a; the h1 PVs are
    # deferred into att(3) via kept-alive PT tiles
    out_a = o_ps.tile([65, 256], F32, tag="o")
    out_b = o_ps.tile([65, 256], F32, tag="o")
    epts = []
    for jj in (2, 3, 4, 5, 6, 7):
        ept = pt_pool.tile([128, 512], BF16)
        st = st_ps.tile([128, 512], F32, tag="st")
        nc.tensor.matmul(st, kT_sb[:, 128 * jj : 128 * (jj + 1)],
                         qT_sb[:, 1536:2048], start=True, stop=True,
                         skip_group_check=True)
        nc.scalar.activation(ept, st, func=EXP, scale=0.125)
        nc.tensor.matmul(out_a, v_sb[:, jj, :], ept[:, 0:256],
                         start=(jj == 2), stop=False, skip_group_check=True)
        epts.append((jj, ept))
    att_emit(3, v_steps(3), [], pre, last_ctx=(out_a, out_b, epts))


_NC = None


def _get_nc():
    global _NC
    if _NC is None:
        _NC = build_bass()
    return _NC


def kernel(x, Wq, Wk, Wv):
    nc = _get_nc()
    wqk_h = np.concatenate([Wq, Wk], axis=1)               # [1024, 128]
    wqk_h = np.ascontiguousarray(
        wqk_h.reshape(8, 128, 128).transpose(1, 0, 2)      # [128, 8, 128]
    ).astype(ml_dtypes.bfloat16)
    wv_h = np.ascontiguousarray(
        Wv.reshape(8, 128, 64).transpose(1, 0, 2)          # [128, 8, 64]
    ).astype(ml_dtypes.bfloat16)
    in_maps = []
    for b in range(8):
        xT = np.ascontiguousarray(x[b].T)                  # [1024, 2048]
        xpk = np.ascontiguousarray(
            xT.reshape(8, 128, 4, 512).transpose(1, 2, 0, 3)  # [128, 4, 8, 512]
        ).astype(ml_dtypes.bfloat16)
        in_maps.append({"xp": xpk, "wqk": wqk_h, "wv": wv_h})
    res = bass_utils.run_bass_kernel_spmd(nc, in_maps, core_ids=list(range(8)))
    out = np.stack([np.ascontiguousarray(res.results[b]["outT"].T)
                    for b in range(8)])
    return out.astype(np.float32)
